# revision 63
# baseline (speedup 1.0000x reference)
# Trainium2 Bass kernel for nn_Attention_43215960932503.
#
# Module: per-head attention over N=56*56=3136 tokens, 8 heads, B=2,
# key_dim=16, v_dim=32, with 1x1-conv+BN projections (BN folded to
# scale+bias) and a final 1x1-conv projection over all heads.
#
# Sharding: 16 (batch, head) pairs over 8 cores -> each core owns one
# batch and two adjacent heads; host sums the 4 partial final
# projections per batch and adds the output bias (exact: linear).
#
# Key optimizations over the fp32 baseline:
#  * all matmuls stream 1-byte/2-byte operands (bf16 = 1 PE cycle/row,
#    fp8e4+DoubleRow = 0.5) instead of fp32 (4 cycles/row)
#  * exp work is split between the ACT engine (true exp -> fp8 P) and
#    the DVE (Schraudolph bit-trick exp: y = S*a + b written as int8 is
#    the bit pattern of fp8 e4m3 exp(S); max ~4% elementwise error,
#    cancels in softmax normalization). ~3.4e-3 end-to-end.
#  * softmax algebra: the key-side bias bk shifts every score of a
#    softmax row equally -> dropped entirely; the query-side bias bq
#    folds into a per-key-position exp bias r_m = k_m^T bq computed by
#    25 tiny matmuls; the fp8-range weight prescale (x8 on Wq,Wk,Wv)
#    folds into exp's scale (1/64) and the rowsum ones-row (8.0), so
#    every PSUM->SBUF projection copy is a pure dtype cast
#  * pure-cast PSUM->SBUF copies are spread across ACT and DVE; the
#    rowsum broadcast is a ones-column matmul on PE
#  * heavy software pipelining via emission order (engines execute
#    in-order): AV matmuls and the normalize chain are deferred into
#    later pairs, chunk j's output projection is emitted inside chunk
#    j+1's stream, and the late prologue blocks are emitted inside
#    chunk 0's m-loop so nothing serializes on the full input DMA
import numpy as np

N = 3136          # tokens = 56*56
NT = 784          # n-chunk (4 chunks, each 2 PSUM banks)
NSUB = ((0, 512), (512, 272))   # matmul free-dim sub-chunks of one n-chunk
MTILES = [(i * 128, 128) for i in range(24)] + [(3072, 64)]  # (offset, rows)
PCHUNK = 448      # projection n-chunk (7 per row)
# Exp engine assignment: most pairs are split member0->ACT (true exp),
# member1->DVE (Schraudolph) so both engines run concurrently on one
# pair; ACT_BOTH pairs rebalance the ratio (DVE also owns the softmax
# normalize + copies).  Tile 24 (unpaired) alternates by (j+h) parity.
ACT_BOTH = (0,)
SCHRAU_A = 8.0 / np.log(2.0)        # fp8e4m3 Schraudolph slope
SCHRAU_B = 56.1                     # 7*8 + 0.5 (trunc->round) - 0.4 (minimax)
WSCALE = 8.0                        # host weight prescale for fp8 range

_CACHE = {}


def _build():
    import concourse.bass as bass
    import concourse.mybir as mybir
    import concourse.tile as tile
    from contextlib import ExitStack

    f32 = mybir.dt.float32
    bf16 = mybir.dt.bfloat16
    f8 = mybir.dt.float8e4
    i8 = mybir.dt.int8
    EXP = mybir.ActivationFunctionType.Exp
    MAX = mybir.AluOpType.max
    MULT = mybir.AluOpType.mult
    ADD = mybir.AluOpType.add
    DR = mybir.MatmulPerfMode.DoubleRow

    nc = bass.Bass()
    x = nc.dram_tensor("x", (256, N), f8, kind="ExternalInput")
    st = nc.dram_tensor("st", (256, N), f8, kind="ExternalInput")
    wqT = nc.dram_tensor("wqT", (128, 2, 2, 16), f8, kind="ExternalInput")
    wkT = nc.dram_tensor("wkT", (128, 2, 2, 16), f8, kind="ExternalInput")
    wvT = nc.dram_tensor("wvT", (128, 2, 64), f8, kind="ExternalInput")
    wpT = nc.dram_tensor("wpT", (32, 2, 256), bf16, kind="ExternalInput")
    bqr = nc.dram_tensor("bqr", (16, 2), bf16, kind="ExternalInput")
    bv = nc.dram_tensor("bv", (1, 64), f8, kind="ExternalInput")
    y = nc.dram_tensor("y", (256, N), f32, kind="ExternalOutput")

    # m-tiles covered by input DMA chunk j (tile inside cols < 784(j+1))
    def vtiles(j):
        lo = 0 if j == 0 else vtiles.hi[j - 1]
        return range(lo, vtiles.hi[j])
    vtiles.hi = [6, 12, 18, 25]
    # k-projection chunks (448 cols) whose x cols arrive with DMA chunk j
    tq = [[0], [1, 2], [3, 4], [5, 6]]
    # m-tiles whose k columns are complete once tq[j]'s projections ran
    rtiles = [range(0, 3), range(3, 10), range(10, 17), range(17, 25)]

    with ExitStack() as ctx:
        tc = ctx.enter_context(tile.TileContext(nc))
        sb = ctx.enter_context(tc.tile_pool(name="sb", bufs=1))
        ptp = ctx.enter_context(tc.tile_pool(name="ptp", bufs=7))
        zp = ctx.enter_context(tc.tile_pool(name="zp", bufs=4))
        yp = ctx.enter_context(tc.tile_pool(name="yp", bufs=2))
        rp = ctx.enter_context(tc.tile_pool(name="rp", bufs=2))
        psa = ctx.enter_context(tc.tile_pool(name="psa", bufs=3, space="PSUM"))
        pso = ctx.enter_context(tc.tile_pool(name="pso", bufs=1, space="PSUM"))

        # ---- persistent SBUF tiles ----
        x_sb = sb.tile([128, 2, N], f8)       # x, dim1 = channel chunk
        st_sb = sb.tile([128, 2, N], f8)
        q_sb = sb.tile([16, 2, N], bf16)      # raw 8*q per head (16, N)
        k_sb = sb.tile([16, 2, N], bf16)
        # [pair][head][member][v|8|pad]: member stride must be a power of
        # two for the DoubleRow ldweights ISA encoding, hence pad to 64
        vT8_sb = sb.tile([128, 13, 2, 2, 64], f8)
        wq_sb = sb.tile([128, 2, 2, 16], f8)   # [p][head][cc][kd]
        wk_sb = sb.tile([128, 2, 2, 16], f8)
        wv_sb = sb.tile([128, 2, 64], f8)
        wp_sb = sb.tile([32, 2, 256], bf16)
        bqr_sb = sb.tile([16, 2, 1], bf16)
        bv_sb = sb.tile([1, 64], f8)
        ones_sb = sb.tile([1, N], f8)
        ones33 = sb.tile([33, 32], bf16)      # fallback rowsum broadcast
        rb_sb = sb.tile([128, 2, 25], f32)    # exp bias r_m = k_m^T bq
        rb2_sb = sb.tile([128, 2, 25], f32)   # schraudolph bias a*r_m + B

        # ---- input DMAs, ordered so the first projections start early:
        # x chunk 0 first (k/v projections), then the weights they need,
        # then st (q projections lag by a chunk), then the rest
        def dmain(sb_t, dram_t, c, j):
            s4 = j * NT
            nc.sync.dma_start(sb_t[:, c, s4:s4 + NT],
                              dram_t[128 * c:128 * (c + 1), s4:s4 + NT])

        dmain(x_sb, x, 0, 0)
        dmain(x_sb, x, 1, 0)
        nc.sync.dma_start(wk_sb[:], wkT[:])
        nc.sync.dma_start(wv_sb[:], wvT[:])
        nc.sync.dma_start(wq_sb[:], wqT[:])
        nc.sync.dma_start(bqr_sb[:, :, 0], bqr[:])
        nc.sync.dma_start(bv_sb[:], bv[:])
        for c in range(2):
            dmain(st_sb, st, c, 0)
            dmain(st_sb, st, c, 1)
        for j in range(1, 4):
            for c in range(2):
                dmain(x_sb, x, c, j)
        nc.sync.dma_start(wp_sb[:], wpT[:])
        for j in range(2, 4):
            for c in range(2):
                dmain(st_sb, st, c, j)
        nc.vector.memset(ones_sb[:], 1.0)
        nc.vector.memset(ones33[:], 1.0)
        nc.vector.memset(vT8_sb[:], WSCALE)  # rowsum cols 32/65 survive as 8

        COPY = mybir.ActivationFunctionType.Copy

        def cast_copy(eng, out_ap, in_ap):
            # pure-dtype-cast PSUM->SBUF copy; ACT is idle in the
            # prologue so alternate it with DVE
            if eng == "act":
                nc.scalar.activation(out=out_ap, in_=in_ap, func=COPY)
            else:
                nc.vector.tensor_copy(out_ap, in_ap)

        # ---- k/v projections + r-bias, per input chunk (q is deferred
        # into the attention stream; blocks 2/3 are emitted INSIDE chunk
        # 0's m-loop so the engines' in-order queues don't serialize the
        # whole input DMA ahead of the first exp) ----
        ncopy = [0]

        def alt():
            ncopy[0] += 1
            return "act" if ncopy[0] % 2 else "dve"

        def emit_prologue(j):
            for t in tq[j]:
                s = t * PCHUNK
                for h in range(2):
                    pk = psa.tile([16, PCHUNK], f32, tag="psa", bufs=3)
                    nc.tensor.matmul(
                        pk[:], wk_sb[:, h], x_sb[:, :, s:s + PCHUNK],
                        perf_mode=DR, start=True, stop=True)
                    cast_copy(alt(), k_sb[:, h, s:s + PCHUNK], pk[:])
            for i in vtiles(j):
                mo, mi = MTILES[i]
                pv = psa.tile([128, 64], f32, tag="psa", bufs=3)
                for c in range(2):
                    nc.tensor.matmul(
                        pv[0:mi, :], x_sb[:, c, mo:mo + mi], wv_sb[:, c, :],
                        start=(c == 0), stop=False)
                nc.tensor.matmul(
                    pv[0:mi, :], ones_sb[:, mo:mo + mi], bv_sb[:],
                    start=False, stop=True)
                out_ap = vT8_sb[0:mi, i // 2, :, i % 2, 0:32]
                in_ap = pv[0:mi, :].rearrange("p (a b) -> p a b", a=2)
                cast_copy(alt(), out_ap, in_ap)
            # r_m = (8k_m)^T (bq/8) for m-tiles whose k is now complete
            for h in range(2):
                pr = psa.tile([128, 25], f32, tag="psa", bufs=3)
                for i in rtiles[j]:
                    mo, mi = MTILES[i]
                    nc.tensor.matmul(
                        pr[0:mi, i:i + 1], k_sb[:, h, mo:mo + mi],
                        bqr_sb[:, h, :], start=True, stop=True)
                i0, i1 = rtiles[j][0], rtiles[j][-1] + 1
                # tile 24 only has 64 rows; don't read the unwritten rest
                parts = ([(i0, 24, 128), (24, 25, 64)] if i1 == 25
                         else [(i0, i1, 128)])
                for (a, b, pp) in parts:
                    nc.vector.tensor_copy(
                        rb_sb[0:pp, h, a:b], pr[0:pp, a:b])
                    nc.vector.tensor_scalar(
                        out=rb2_sb[0:pp, h, a:b], in0=pr[0:pp, a:b],
                        scalar1=SCHRAU_A, scalar2=SCHRAU_B, op0=MULT, op1=ADD)

        emit_prologue(0)
        emit_prologue(1)

        # ---- attention; chunk j's output projection emitted inside j+1 ----
        zs = {}

        def emit_qproj(j):
            for t in ([0, 1], [2, 3], [4, 5], [6])[j]:
                s = t * PCHUNK
                for h in range(2):
                    pq = psa.tile([16, PCHUNK], f32, tag="psa", bufs=3)
                    nc.tensor.matmul(
                        pq[:], wq_sb[:, h], st_sb[:, :, s:s + PCHUNK],
                        perf_mode=DR, start=True, stop=True)
                    nc.scalar.activation(out=q_sb[:, h, s:s + PCHUNK], in_=pq[:], func=COPY)

        proj_py = {}

        def emit_proj(j, subs=NSUB, last=True, ocs=(0, 1)):
            jc = j * NT
            for oc in ocs:
                if (j, oc) not in proj_py:
                    proj_py[(j, oc)] = psa.tile(
                        [128, NT], f32, tag="psa", bufs=3, name=f"py{j}{oc}")
                py = proj_py[(j, oc)]
                for (o, w) in subs:
                    for h in range(2):
                        nc.tensor.matmul(
                            py[:, o:o + w],
                            wp_sb[:, h, 128 * oc:128 * (oc + 1)],
                            zs[(j, h)][:, o:o + w],
                            start=(h == 0), stop=(h == 1))
                if last:
                    y_sb = yp.tile([128, NT], f32, tag="y")
                    cast_copy(alt(), y_sb[:], py[:])
                    nc.sync.dma_start(
                        y[128 * oc:128 * (oc + 1), jc:jc + NT], y_sb[:])

        emit_qproj(0)
        norm_tail = {}

        for j in range(4):
            jc = j * NT
            for h in range(2):
                po = pso.tile([33, NT], f32, tag="pso", bufs=1)

                def qk(i, dst):
                    mo, mi = MTILES[i]
                    for (o, w) in NSUB:
                        nc.tensor.matmul(
                            dst[0:mi, o:o + w], k_sb[:, h, mo:mo + mi],
                            q_sb[:, h, jc + o:jc + o + w],
                            start=True, stop=True)

                def expi(eng, i, out_ap):
                    ps = psa.tile([128, NT], f32, tag="psa", bufs=3)
                    qk(i, ps)
                    mi = MTILES[i][1]
                    if eng == "act":
                        nc.scalar.activation(
                            out=out_ap, in_=ps[0:mi, :], func=EXP,
                            scale=1.0 / (WSCALE * WSCALE),
                            bias=rb_sb[0:mi, h, i:i + 1])
                    else:
                        nc.vector.tensor_scalar(
                            out=out_ap.bitcast(i8), in0=ps[0:mi, :],
                            scalar1=SCHRAU_A / (WSCALE * WSCALE),
                            scalar2=rb2_sb[0:mi, h, i:i + 1],
                            op0=MULT, op1=ADD)

                # AV(p) is emitted several pairs late so the in-order PE
                # never waits on exp results (ps WAR) or the po slot (stt
                # of the previous head) before issuing the next QKs
                def av(p, pt, start=False, stop=False):
                    for (o, w) in NSUB:
                        nc.tensor.matmul(
                            po[:, o:o + w], vT8_sb[:, p, h, :, 0:33],
                            pt[:, :, o:o + w], perf_mode=DR,
                            start=start, stop=stop)

                def av24(pt24, start=False, stop=False):
                    m24 = MTILES[24][1]
                    for (o, w) in NSUB:
                        nc.tensor.matmul(
                            po[:, o:o + w], vT8_sb[0:m24, 12, h, 0, 0:33],
                            pt24[0:m24, 0, o:o + w], start=start, stop=stop)

                # mid-loop emission hooks: the previous block's normalize
                # tail (broadcast matmul would otherwise stall PE at the
                # boundary), late prologue blocks (chunk 0), and the
                # previous chunk's output projection split per half
                hooks = {1: lambda: [f() for f in norm_tail.pop(0, [])]}
                if j == 0 and h == 0:
                    hooks[4] = lambda: emit_prologue(2)
                    hooks[7] = lambda: emit_prologue(3)
                if j > 0 and h == 0:
                    hooks[6] = lambda: emit_proj(j - 1, ocs=(0,))
                    hooks[9] = lambda: emit_proj(j - 1, ocs=(1,))

                # the unpaired tile 24 runs FIRST (except (0,h0), whose
                # prologue block for it lands mid-loop) so no serial
                # exp24->AV24 chain dangles at the block boundary
                defer = 2 if (j, h) == (3, 1) else 4
                t24_first = not (j == 0 and h == 0)
                e24 = "act" if (j + h) % 2 else "dve"
                if t24_first:
                    # exp of tile 24 issues early (warms ACT/DVE); its AV
                    # is held back until the first deferred AV so it never
                    # waits on the previous block's deferred normalize
                    pt24 = ptp.tile([128, 2, NT], f8, tag="pt")
                    expi(e24, 24, pt24[0:MTILES[24][1], 0, :])
                pend = []
                for p in range(12):
                    if p in hooks:
                        hooks[p]()
                    pt = ptp.tile([128, 2, NT], f8, tag="pt")
                    e1 = "act" if p in ACT_BOTH else "dve"
                    expi("act", 2 * p, pt[:, 0, :])
                    expi(e1, 2 * p + 1, pt[:, 1, :])
                    pend.append(pt)
                    if len(pend) > defer:
                        if p == defer and t24_first:
                            av24(pt24, start=True)
                        av(p - defer, pend.pop(0),
                           start=(p == defer and not t24_first))
                for pi, pt_ in enumerate(pend):
                    last = pi == len(pend) - 1
                    av(12 - len(pend) + pi, pt_, stop=last and t24_first)
                if not t24_first:
                    pt24 = ptp.tile([128, 2, NT], f8, tag="pt")
                    expi(e24, 24, pt24[0:MTILES[24][1], 0, :])
                    av24(pt24, stop=True)

                # normalize: rowsum copy now; the broadcast matmul /
                # reciprocal / relu*mul are deferred into the next block's
                # stream (hook p=1) so PE isn't stalled at the boundary.
                # The last (j,h) runs everything inline per NSUB half so
                # the final output projection overlaps the chain.
                z = zp.tile([32, NT], bf16, tag="z")
                zs[(j, h)] = z
                subs = NSUB if (j, h) == (3, 1) else ((0, NT),)
                for (o, w) in subs:
                    r1 = rp.tile([33, NT], bf16, tag="rc")
                    # rowsum copy on ACT: DVE is the busier engine here
                    nc.scalar.activation(out=r1[32:33, 0:w], in_=po[32:33, o:o + w], func=COPY)

                    def ntail(po=po, z=z, r1=r1, o=o, w=w):
                        pbc = psa.tile([32, NT], f32, tag="psa", bufs=3)
                        for (o2, w2) in (NSUB if w == NT else ((0, w),)):
                            nc.tensor.matmul(
                                pbc[:, o2:o2 + w2], ones33[32:33, 0:32],
                                r1[32:33, o2:o2 + w2] if w == NT
                                else r1[32:33, 0:w],
                                start=True, stop=True)
                        rbc = rp.tile([32, NT], f32, tag="rbc")
                        nc.vector.reciprocal(rbc[:, 0:w], pbc[:, 0:w])
                        nc.vector.scalar_tensor_tensor(
                            out=z[:, o:o + w], in0=po[0:32, o:o + w],
                            scalar=0.0, in1=rbc[:, 0:w], op0=MAX, op1=MULT)

                    if (j, h) == (3, 1):
                        ntail()
                        emit_proj(3, subs=((o, w),), last=(o + w == NT))
                    else:
                        norm_tail[0] = norm_tail.get(0, []) + [ntail]
                # q projection for the next chunk sits after the normalize
                # chain (its PE matmuls land behind the AV flush, exactly
                # when ACT drains its exp backlog)
                if h == 0 and j < 3:
                    emit_qproj(j + 1)

    return nc


def _prep_in_maps(x, singlex, Wq, sq, bq, Wk, sk, bk, Wv, sv, bv, Wp, sp, bp):
    import ml_dtypes
    bf = ml_dtypes.bfloat16
    f8 = ml_dtypes.float8_e4m3
    xf = np.ascontiguousarray(x.reshape(2, 256, N), dtype=f8)
    sf = np.ascontiguousarray(singlex.reshape(2, 256, N), dtype=f8)
    Wq_s = WSCALE * sq[:, None] * Wq
    Wk_s = WSCALE * sk[:, None] * Wk
    Wv_s = WSCALE * sv[:, None] * Wv
    Wp_s = sp[:, None] * Wp
    in_maps = []
    for c in range(8):
        b, hp = c // 4, c % 4
        g0, g1 = 2 * hp, 2 * hp + 1
        # (128, 2, 2, rows): [p, h, cc, r] = W_s[rows*g_h + r, 128 cc + p]
        def wmap(W, rows):
            out = np.empty((128, 2, 2, rows), dtype=np.float32)
            for hh, g in enumerate((g0, g1)):
                blk = W[rows * g:rows * g + rows]      # (rows, 256)
                out[:, hh, 0, :] = blk[:, 0:128].T
                out[:, hh, 1, :] = blk[:, 128:256].T
            return out

        # wv keeps [p, cc, dv] (used by plain per-chunk matmuls)
        def wvmap(W, rows):
            out = np.empty((128, 2, 2 * rows), dtype=np.float32)
            for hh, g in enumerate((g0, g1)):
                blk = W[rows * g:rows * g + rows]
                out[:, 0, rows * hh:rows * hh + rows] = blk[:, 0:128].T
                out[:, 1, rows * hh:rows * hh + rows] = blk[:, 128:256].T
            return out
        in_maps.append({
            "x": xf[b],
            "st": sf[b],
            "wqT": np.ascontiguousarray(wmap(Wq_s, 16), dtype=f8),
            "wkT": np.ascontiguousarray(wmap(Wk_s, 16), dtype=f8),
            "wvT": np.ascontiguousarray(wvmap(Wv_s, 32), dtype=f8),
            "wpT": np.ascontiguousarray(
                np.stack([Wp_s[:, 32 * g0:32 * g0 + 32].T,
                          Wp_s[:, 32 * g1:32 * g1 + 32].T], 1), dtype=bf),
            "bqr": np.ascontiguousarray(
                np.stack([bq[16 * g0:16 * g0 + 16],
                          bq[16 * g1:16 * g1 + 16]], 1) / WSCALE, dtype=bf),
            "bv": np.ascontiguousarray(
                np.concatenate([WSCALE * bv[32 * g0:32 * g0 + 32],
                                WSCALE * bv[32 * g1:32 * g1 + 32]])[None, :],
                dtype=f8),
        })
    return in_maps


def _fix_bir(bir_json):
    # This toolchain's walrus accepts only ONE sync-wait per instruction
    # on several instruction structs (Matmult/LDWEIGHTS, Drain, ...).
    # Engines execute in order, so any excess waits can be hoisted onto
    # inserted same-engine NoOps immediately before the instruction.
    import json as _json
    j = _json.loads(bir_json)
    cnt = [0]

    def fix_block(bk):
        out = []
        for ins in bk.get("instructions", []):
            si = ins.get("sync_info")
            if si and si.get("on_wait") and len(si["on_wait"]) > 1:
                waits = si["on_wait"]
                for w in waits[:-1]:
                    cnt[0] += 1
                    out.append({
                        "debug": ins.get("debug"), "engine": ins["engine"],
                        "ins": [], "name": f"I-wfix-{cnt[0]}",
                        "opcode": "NoOp", "outs": [],
                        "sync_info": {"on_update": [], "on_wait": [w]}})
                si["on_wait"] = [waits[-1]]
            out.append(ins)
        bk["instructions"] = out
        for sbk in bk.get("blocks", []):
            fix_block(sbk)

    for f in j["functions"]:
        for bk in f["blocks"]:
            fix_block(bk)
    return _json.dumps(j).encode()


def _patch_compiler():
    if _CACHE.get("patched"):
        return
    import concourse.bass_utils as bu
    import concourse.bass2jax as b2j
    orig = bu.compile_bir_kernel

    def patched(bir_json, tmpdir, neff_name="file.neff"):
        return orig(_fix_bir(bir_json), tmpdir, neff_name)

    bu.compile_bir_kernel = patched
    if getattr(b2j, "compile_bir_kernel", None) is orig:
        b2j.compile_bir_kernel = patched
    _CACHE["patched"] = True


def run(trace=False, **inputs):
    from concourse.bass_utils import run_bass_kernel_spmd

    _patch_compiler()
    inputs = {k: np.asarray(v) for k, v in inputs.items()}
    if "nc" not in _CACHE:
        _CACHE["nc"] = _build()
    in_maps = _prep_in_maps(**inputs)
    res = run_bass_kernel_spmd(
        _CACHE["nc"], in_maps, core_ids=list(range(8)), trace=trace)
    bp = inputs["bp"].astype(np.float32)
    out = np.zeros((2, 256, N), dtype=np.float32)
    for c in range(8):
        out[c // 4] += res.results[c]["y"]
    out += bp[None, :, None]
    return out.reshape(2, 256, 56, 56), res


def kernel(**inputs):
    return run(**inputs)[0]
